# revision 1
# baseline (speedup 1.0000x reference)

# Trainium2 Bass kernel for MinConvExpLSTMCell.
#
# Math (linear-space reformulation of the reference's log-space scan):
#   y = conv3x3(x, W) + b; [f_gate, i_gate, h_tilde] = split(y)
#   diff = f_gate - i_gate = conv(x, W_f - W_i) + (b_f - b_i)
#   f = sigmoid(diff);  i = 1 - f
#   g = sigmoid(min(ht, 0)) + relu(ht)          (= g(h_tilde), exact identity)
#   h_t = f_t * h_{t-1} + i_t * g_t,  h_{-1} = g(h0)
#
# Sharding: 8 cores = 4 batches x 2 spatial halves (16 output rows each,
# 1 halo row). Conv = 9 accumulated matmuls per time step (K=64 in-ch,
# M=128 = [diff;ht] out-ch, N=512 px), bf16, row-tiled in pairs across PE
# row-groups (image duplicated on partitions 0-63 / 64-127).
# Recurrence: tensor_tensor_scan along a pixel-major/time-minor layout,
# segmented by 8 time steps, chained via a per-pixel init column.

import sys
import numpy as np

sys.path.insert(0, "/opt/trn_rl_repo")

import ml_dtypes
from contextlib import ExitStack

import concourse.bass as bass
import concourse.bacc as bacc
import concourse.mybir as mybir
from concourse.tile import TileContext
from concourse.bass_utils import run_bass_kernel_spmd

BF16 = ml_dtypes.bfloat16
B, T, C, H, W = 4, 64, 64, 32, 32
SEG = 8
NSEG = T // SEG
HP, WP = 18, 34            # padded shard rows/cols
RC = HP * WP               # 612
NPX = 16 * 32              # 512 output pixels per core
TS = SEG + 1               # 9 scan slots per pixel per segment
NF = NPX * TS              # 4608 scan free size
TAPS = [(r0, c0) for r0 in range(3) for c0 in range(3)]

_CACHE = {}


def _build():
    f32 = mybir.dt.float32
    bf16 = mybir.dt.bfloat16
    AF = mybir.ActivationFunctionType
    OP = mybir.AluOpType

    nc = bacc.Bacc()
    xs = nc.dram_tensor("xs", [T, C, RC], bf16, kind="ExternalInput")
    wt = nc.dram_tensor("wt", [128, 9 * 128], bf16, kind="ExternalInput")
    cst = nc.dram_tensor("cst", [64, 2 + NPX], f32, kind="ExternalInput")
    out = nc.dram_tensor("out", [NSEG, 64, NF], f32, kind="ExternalOutput")

    with TileContext(nc) as tc, ExitStack() as ctx:
        cpool = ctx.enter_context(tc.tile_pool(name="consts", bufs=1))
        xpool = ctx.enter_context(tc.tile_pool(name="x", bufs=2))
        pspool = ctx.enter_context(tc.tile_pool(name="ps", bufs=2, space="PSUM"))
        gpool = ctx.enter_context(tc.tile_pool(name="g", bufs=2))
        sigpool = ctx.enter_context(tc.tile_pool(name="sig", bufs=2))
        spool = ctx.enter_context(tc.tile_pool(name="s", bufs=1))
        rpool = ctx.enter_context(tc.tile_pool(name="r", bufs=1))
        ggpool = ctx.enter_context(tc.tile_pool(name="gg", bufs=1))
        ipool = ctx.enter_context(tc.tile_pool(name="i", bufs=1))
        upool = ctx.enter_context(tc.tile_pool(name="u", bufs=2))
        hpool = ctx.enter_context(tc.tile_pool(name="h", bufs=2))

        w_sb = cpool.tile([128, 9 * 128], bf16)
        nc.sync.dma_start(w_sb[:, :], wt[:, :])
        cst_sb = cpool.tile([64, 2 + NPX], f32)
        nc.sync.dma_start(cst_sb[:, :], cst[:, :])
        bd = cst_sb[:, 0:1]
        bh = cst_sb[:, 1:2]
        g0 = cst_sb[:, 2:2 + NPX]

        h_prev = None
        for s in range(NSEG):
            xt = xpool.tile([64, SEG * RC], bf16)
            src = xs[s * SEG:(s + 1) * SEG].rearrange("t c x -> c t x")
            nc.sync.dma_start(
                xt[0:64, :].rearrange("p (t x) -> p t x", t=SEG), src)
            xv = xt.rearrange("p (t r c) -> p t r c", t=SEG, r=HP, c=WP)

            G = gpool.tile([128, SEG * 512], bf16)
            for k in range(SEG):
                ps = pspool.tile([128, 512], f32)
                for j, (r0, c0) in enumerate(TAPS):
                    rhs = xv[0:64, k, r0:r0 + 16, c0:c0 + 32]
                    lhsT = w_sb[0:64, j * 128:(j + 1) * 128]
                    nc.tensor.matmul(
                        ps[:, :], lhsT, rhs,
                        start=(j == 0), stop=(j == 8))
                dst = G[:, k * 512:(k + 1) * 512]
                if k % 2 == 0:
                    nc.scalar.activation(dst, ps[:, :], AF.Copy)
                else:
                    nc.vector.tensor_copy(dst, ps[:, :])

            Gd = G[0:64, :]
            Gh = G[64:128, :]

            # r = relu(ht + bh)   (before min destroys Gh)
            R = rpool.tile([64, SEG * 512], bf16)
            nc.gpsimd.tensor_scalar(R[:, :], Gh, bh, 0.0, OP.add, OP.max)
            # Gh <- min(ht + bh, 0)  in place
            nc.vector.tensor_scalar(Gh, Gh, bh, 0.0, OP.add, OP.min)

            # f = sigmoid(diff + bd) -> fp32, scan layout (t-minor)
            SIG = sigpool.tile([64, NF], f32)
            sig9 = SIG.rearrange("p (px t) -> p t px", t=TS)
            nc.gpsimd.memset(sig9[:, 0, :], 0.0)  # scan-reset column
            nc.scalar.activation(sig9[:, 1:TS, :], Gd, AF.Sigmoid, bias=bd)

            # s = sigmoid(min(ht+bh,0)) -> bf16 dense (t-major)
            S = spool.tile([64, SEG * 512], bf16)
            nc.scalar.activation(S[:, :], Gh, AF.Sigmoid)

            # g = s + r -> scan layout
            GG = ggpool.tile([64, NF], bf16)
            gg9 = GG.rearrange("p (px t) -> p t px", t=TS)
            nc.gpsimd.tensor_tensor(gg9[:, 1:TS, :], S[:, :], R[:, :], OP.add)

            # i = 1 - f (dense over scan buffer; col0 junk unused)
            I = ipool.tile([64, NF], bf16)
            nc.vector.tensor_scalar(
                I[:, :], SIG[:, :], -1.0, 1.0, OP.mult, OP.add)

            # u = i * g -> fp32 scan layout (skip col0)
            U = upool.tile([64, NF], f32)
            u9 = U.rearrange("p (px t) -> p px t", t=TS)
            i9 = I.rearrange("p (px t) -> p px t", t=TS)
            gx9 = GG.rearrange("p (px t) -> p px t", t=TS)
            nc.gpsimd.tensor_tensor(
                u9[:, :, 1:TS], i9[:, :, 1:TS], gx9[:, :, 1:TS], OP.mult)

            # u col0 = h_{-1} for this segment (chains segments)
            if h_prev is None:
                nc.vector.tensor_copy(u9[:, :, 0], g0)
            else:
                hp9 = h_prev.rearrange("p (px t) -> p px t", t=TS)
                nc.vector.tensor_copy(u9[:, :, 0], hp9[:, :, SEG])

            # h = scan: state = (f * state) + u, per-pixel chains
            Ht = hpool.tile([64, NF], f32)
            nc.vector.tensor_tensor_scan(
                Ht[:, :], SIG[:, :], U[:, :], 0.0, OP.mult, OP.add)
            h_prev = Ht

            nc.sync.dma_start(out[s], Ht[:, :])
    nc.finalize()
    return nc


def _g0(h0):
    return np.where(h0 >= 0.0, h0 + 0.5, 1.0 / (1.0 + np.exp(-h0))).astype(np.float32)


def kernel(x, conv_w, conv_b, h0):
    x = np.asarray(x, np.float32)
    conv_w = np.asarray(conv_w, np.float32)
    conv_b = np.asarray(conv_b, np.float32)
    h0 = np.asarray(h0, np.float32)

    if "nc" not in _CACHE:
        _CACHE["nc"] = _build()
    nc = _CACHE["nc"]

    wd = conv_w[0:64] - conv_w[64:128]
    wh = conv_w[128:192]
    wcat = np.concatenate([wd, wh], 0)           # [128 out, 64 in, 3, 3]
    bd = conv_b[0:64] - conv_b[64:128]
    bh = conv_b[128:192]

    wt = np.zeros((128, 9 * 128), np.float32)
    for j, (r0, c0) in enumerate(TAPS):
        # lhsT[k, m] = wcat[m, k, r0, c0]
        wt[0:64, j * 128:(j + 1) * 128] = wcat[:, :, r0, c0].T
    wt = wt.astype(BF16)

    x4 = x.reshape(B, T, C, H, W)
    g0f = _g0(h0)                                 # [B, C, H, W]

    in_maps = []
    for c in range(8):
        b, half = c // 2, c % 2
        xsh = np.zeros((T, C, HP, WP), np.float32)
        if half == 0:
            xsh[:, :, 1:18, 1:33] = x4[b, :, :, 0:17, :]
        else:
            xsh[:, :, 0:17, 1:33] = x4[b, :, :, 15:32, :]
        xsh = xsh.reshape(T, C, RC).astype(BF16)
        g0c = g0f[b, :, 16 * half:16 * half + 16, :].reshape(64, NPX)
        cst = np.concatenate(
            [bd[:, None], bh[:, None], g0c], 1).astype(np.float32)
        in_maps.append({"xs": xsh, "wt": wt, "cst": cst})

    _CACHE["in_maps"] = in_maps
    res = run_bass_kernel_spmd(nc, in_maps, core_ids=list(range(8)))

    outf = np.empty((B, T, C, H, W), np.float32)
    for c in range(8):
        b, half = c // 2, c % 2
        arr = res.results[c]["out"]               # [NSEG, 64, NF]
        hseq = arr.reshape(NSEG, 64, NPX, TS)[:, :, :, 1:]
        hseq = hseq.transpose(0, 3, 1, 2).reshape(T, C, 16, 32)
        outf[b, :, :, 16 * half:16 * half + 16, :] = hseq
    return outf.reshape(B * T, C, H, W)



# revision 2
# speedup vs baseline: 4.7397x; 4.7397x over previous

# Trainium2 Bass kernel for MinConvExpLSTMCell (v2).
#
# Math (linear-space reformulation of the reference's log-space scan):
#   y = conv3x3(x, W) + b; [f_gate, i_gate, h_tilde] = split(y)
#   diff = f_gate - i_gate = conv(x, W_f - W_i) + (b_f - b_i)
#   f = sigmoid(diff);  i = 1 - f = sigmoid(-diff)
#   g = sigmoid(min(y,0)) + relu(y),  y = h_tilde + b_h   (exact identity)
#   h_t = f_t * h_{t-1} + i_t * g_t,  h_{-1} = g(h0)
#
# Sharding: 8 cores = 4 batches x 2 spatial halves (16 output rows each).
#
# Matmul: K=128 tap-pair packing - x is stored twice in SBUF (partitions
# 0:63 normal "copy A", partitions 64:127 shifted down one image row
# "copy B"), so one K=128 matmul contracts two vertically-adjacent taps
# at once: taps (0,c)+(1,c) paired, taps (2,c) single (zero weights in
# rows 64:127). 6 phases x 2 outputs (diff, ht) x 2-step pairs (N=512).
# 2x column tiling splits each output into px-lo (psum partitions 0:63)
# and px-hi (64:127), so ALL post-processing runs on 128 partitions.
#
# Post (per 4-step half / 8-step segment): ACT does sigmoid(f),
# sigmoid(-diff)=i, y=ht+bh; DVE does min/max/g/u and the
# tensor_tensor_scan (per-pixel chains, 9 slots: 1 init + 8 steps).

import sys
import numpy as np

sys.path.insert(0, "/opt/trn_rl_repo")

import ml_dtypes
from contextlib import ExitStack

import concourse.bass as bass
import concourse.bacc as bacc
import concourse.mybir as mybir
from concourse.tile import TileContext
from concourse.bass_utils import run_bass_kernel_spmd

BF16 = ml_dtypes.bfloat16
B, T, C, H, W = 4, 64, 64, 32, 32
SEG = 8
NSEG = T // SEG
HP, WP = 18, 34            # padded shard rows/cols
RC = HP * WP               # 612
RCE = RC + WP              # 646: one extra zero row for shifted copy B
PXH = 256                  # pixels per column-strip (8 rows x 32 cols)
TS = SEG + 1               # scan slots per pixel per segment
NF = PXH * TS              # 2304 scan free size
DNS = SEG * PXH            # 2048 dense free size

_CACHE = {}


def _build():
    f32 = mybir.dt.float32
    bf16 = mybir.dt.bfloat16
    AF = mybir.ActivationFunctionType
    OP = mybir.AluOpType

    nc = bacc.Bacc()
    xs = nc.dram_tensor("xs", [T, C, RCE], bf16, kind="ExternalInput")
    wt = nc.dram_tensor("wt", [128, 768], bf16, kind="ExternalInput")
    cst = nc.dram_tensor("cst", [128, 3 + PXH], f32, kind="ExternalInput")
    out = nc.dram_tensor("out", [NSEG, 128, NF], f32, kind="ExternalOutput")

    with TileContext(nc) as tc, ExitStack() as ctx:
        cpool = ctx.enter_context(tc.tile_pool(name="consts", bufs=1))
        xpool = ctx.enter_context(tc.tile_pool(name="x", bufs=3))
        pspool = ctx.enter_context(tc.tile_pool(name="ps", bufs=2, space="PSUM"))
        sigpool = ctx.enter_context(tc.tile_pool(name="sig", bufs=2))
        ypool = ctx.enter_context(tc.tile_pool(name="y", bufs=2))
        mpool = ctx.enter_context(tc.tile_pool(name="m", bufs=2))
        rpool = ctx.enter_context(tc.tile_pool(name="r", bufs=2))
        spool = ctx.enter_context(tc.tile_pool(name="s", bufs=2))
        gpool = ctx.enter_context(tc.tile_pool(name="g", bufs=2))
        ipool = ctx.enter_context(tc.tile_pool(name="i", bufs=2))
        upool = ctx.enter_context(tc.tile_pool(name="u", bufs=2))
        hpool = ctx.enter_context(tc.tile_pool(name="h", bufs=3))

        w_sb = cpool.tile([128, 768], bf16)
        nc.sync.dma_start(w_sb[:, :], wt[:, :])
        cst_sb = cpool.tile([128, 3 + PXH], f32)
        nc.sync.dma_start(cst_sb[:, :], cst[:, :])
        bd2 = cst_sb[:, 0:1]
        bh2 = cst_sb[:, 1:2]
        nbd2 = cst_sb[:, 2:3]
        g0c = cst_sb[:, 3:3 + PXH]

        h_prev = None
        for s in range(NSEG):
            xt = xpool.tile([128, SEG * RC], bf16)
            xtv = xt.rearrange("p (t x) -> p t x", t=SEG)
            srcA = xs[s * SEG:(s + 1) * SEG, :, 0:RC].rearrange(
                "t c x -> c t x")
            nc.sync.dma_start(xtv[0:64], srcA)
            srcB = xs[s * SEG:(s + 1) * SEG, :, WP:WP + RC].rearrange(
                "t c x -> c t x")
            nc.sync.dma_start(xtv[64:128], srcB)
            xv = xt.rearrange("p (t r c) -> p t r c", t=SEG, r=HP, c=WP)

            SIGt = sigpool.tile([128, NF], f32)
            Ut = upool.tile([128, NF], f32)
            Ht = hpool.tile([128, NF], f32)
            Yt = ypool.tile([128, DNS], bf16)
            Mt = mpool.tile([128, DNS], bf16)
            Rt = rpool.tile([128, DNS], bf16)
            St = spool.tile([128, DNS], bf16)
            Gt = gpool.tile([128, DNS], bf16)
            It = ipool.tile([128, DNS], bf16)
            sig9 = SIGt.rearrange("p (x t) -> p t x", t=TS)
            u9 = Ut.rearrange("p (x t) -> p t x", t=TS)

            # scan-restart column: f=0 at slot 0 of every pixel
            nc.vector.memset(sig9[:, 0, :], 0.0)

            for hf in range(2):
                ps = pspool.tile([128, 2048], f32)
                for kappa in range(2):           # 0 = diff, 1 = h_tilde
                    po = kappa * 1024
                    for p in range(6):
                        blk = (kappa * 6 + p) * 64
                        lhsT = w_sb[:, blk:blk + 64]
                        r0, c0 = (0, p) if p < 3 else (2, p - 3)
                        for sp in range(2):
                            t0 = hf * 4 + sp * 2
                            for cs_ in range(2):
                                rhs = xv[0:128, t0:t0 + 2,
                                         r0 + 8 * cs_:r0 + 8 * cs_ + 8,
                                         c0:c0 + 32]
                                dst = ps[cs_ * 64:(cs_ + 1) * 64,
                                         po + sp * 512:po + sp * 512 + 512]
                                nc.tensor.matmul(
                                    dst, lhsT, rhs,
                                    start=(p == 0), stop=(p == 5),
                                    tile_position=(0, cs_ * 64))

                pdv = ps[:, 0:1024].rearrange("p (t x) -> p t x", t=4)
                # f = sigmoid(diff + bd), written into scan slots
                nc.scalar.activation(
                    sig9[:, 1 + hf * 4:5 + hf * 4, :], pdv,
                    AF.Sigmoid, bias=bd2)
                # i = 1 - f = sigmoid(-diff - bd)
                nc.scalar.activation(
                    It[:, hf * 1024:(hf + 1) * 1024], ps[:, 0:1024],
                    AF.Sigmoid, bias=nbd2, scale=-1.0)
                # y = ht + bh
                nc.scalar.activation(
                    Yt[:, hf * 1024:(hf + 1) * 1024], ps[:, 1024:2048],
                    AF.Identity, bias=bh2)
                yh = Yt[:, hf * 1024:(hf + 1) * 1024]
                nc.vector.tensor_scalar(
                    Mt[:, hf * 1024:(hf + 1) * 1024], yh, 0.0, None, OP.min)
                nc.vector.tensor_scalar(
                    Rt[:, hf * 1024:(hf + 1) * 1024], yh, 0.0, None, OP.max)

            # s = sigmoid(min(y,0)); g = s + relu(y); u = i * g
            nc.scalar.activation(St[:, :], Mt[:, :], AF.Sigmoid)
            nc.vector.tensor_tensor(Gt[:, :], St[:, :], Rt[:, :], OP.add)
            iv = It.rearrange("p (t x) -> p t x", t=SEG)
            gv = Gt.rearrange("p (t x) -> p t x", t=SEG)
            nc.vector.tensor_tensor(u9[:, 1:TS, :], iv, gv, OP.mult)

            # u slot0 = h_{-1} for this segment (chains segments)
            if h_prev is None:
                nc.vector.tensor_copy(u9[:, 0, :], g0c)
            else:
                hp9 = h_prev.rearrange("p (x t) -> p t x", t=TS)
                nc.vector.tensor_copy(u9[:, 0, :], hp9[:, SEG, :])

            # h = scan: state = f * state + u, per-pixel chains
            nc.vector.tensor_tensor_scan(
                Ht[:, :], SIGt[:, :], Ut[:, :], 0.0, OP.mult, OP.add)
            h_prev = Ht

            nc.sync.dma_start(out[s], Ht[:, :])
    nc.finalize()
    return nc


def _g0(h0):
    return np.where(h0 >= 0.0, h0 + 0.5, 1.0 / (1.0 + np.exp(-h0))).astype(np.float32)


def kernel(x, conv_w, conv_b, h0):
    x = np.asarray(x, np.float32)
    conv_w = np.asarray(conv_w, np.float32)
    conv_b = np.asarray(conv_b, np.float32)
    h0 = np.asarray(h0, np.float32)

    if "nc" not in _CACHE:
        _CACHE["nc"] = _build()
    nc = _CACHE["nc"]

    wd = conv_w[0:64] - conv_w[64:128]           # [64out, 64in, 3, 3]
    wh = conv_w[128:192]
    bd = conv_b[0:64] - conv_b[64:128]
    bh = conv_b[128:192]

    wt = np.zeros((128, 768), np.float32)
    for kappa, wk in ((0, wd), (1, wh)):
        for p in range(6):
            blk = (kappa * 6 + p) * 64
            if p < 3:
                # paired taps (0,p) on rows 0:64, (1,p) on rows 64:128
                wt[0:64, blk:blk + 64] = wk[:, :, 0, p].T
                wt[64:128, blk:blk + 64] = wk[:, :, 1, p].T
            else:
                # single tap (2, p-3); rows 64:128 stay zero
                wt[0:64, blk:blk + 64] = wk[:, :, 2, p - 3].T
    wt = wt.astype(BF16)

    x4 = x.reshape(B, T, C, H, W)
    g0f = _g0(h0)                                 # [B, C, H, W]

    bd2 = np.concatenate([bd, bd])[:, None]
    bh2 = np.concatenate([bh, bh])[:, None]

    in_maps = []
    for c in range(8):
        b, half = c // 2, c % 2
        xsh = np.zeros((T, C, HP + 1, WP), np.float32)
        if half == 0:
            xsh[:, :, 1:18, 1:33] = x4[b, :, :, 0:17, :]
        else:
            xsh[:, :, 0:17, 1:33] = x4[b, :, :, 15:32, :]
        xsh = xsh.reshape(T, C, RCE).astype(BF16)
        r16 = g0f[b, :, 16 * half:16 * half + 16, :]     # [64, 16, 32]
        g0c = np.concatenate(
            [r16[:, 0:8, :].reshape(64, PXH),
             r16[:, 8:16, :].reshape(64, PXH)], 0)       # [128, 256]
        cstc = np.concatenate(
            [bd2, bh2, -bd2, g0c], 1).astype(np.float32)
        in_maps.append({"xs": xsh, "wt": wt, "cst": cstc})

    _CACHE["in_maps"] = in_maps
    res = run_bass_kernel_spmd(nc, in_maps, core_ids=list(range(8)))

    outf = np.empty((B, T, C, H, W), np.float32)
    for c in range(8):
        b, half = c // 2, c % 2
        arr = res.results[c]["out"]               # [NSEG, 128, NF]
        arr = arr.reshape(NSEG, 128, PXH, TS)[:, :, :, 1:]
        arr = arr.transpose(0, 3, 1, 2).reshape(T, 128, 8, 32)
        outf[b, :, :, 16 * half:16 * half + 8, :] = arr[:, 0:64]
        outf[b, :, :, 16 * half + 8:16 * half + 16, :] = arr[:, 64:128]
    return outf.reshape(B * T, C, H, W)
